# revision 1
# baseline (speedup 1.0000x reference)
"""Trainium2 Bass kernel for the confidence-based contrastive loss.

Distribution (8 NeuronCores, SPMD):
  - Pixel grid (H*W = 262144) sharded 8-ways by flat index; each core owns
    32768 pixels of the image, staged pixel-major [32768, 256] in its HBM.
  - Each core dma_gathers its "core-confidence" pixels (g/b classes), computes
    1/||x|| per pixel and accumulates the masked, normalized per-class mean
    via PE matmuls.  The [128,4] per-class mean partials are combined with the
    only collective in the kernel (tiny AllReduce).
  - The sampled anchor sets (4096 g + 4096 b) are extracted and normalized on
    the host (host already owns the data-dependent sampling plan, exactly as
    the reference's host-side _plan does) and replicated channel-major to all
    cores.  Each core computes sim = anchors[i-slice] x all-negatives on PE
    (fp32), exp(sim/tau) on ACT, per-100-chunk sums on DVE (segmented
    reduce), then log(1 + S*exp(-pos/tau)) and the per-anchor loss partials.
  - Host sums the 8x[128,2] partials -> scalar loss.
"""

import sys

if "/opt/trn_rl_repo" not in sys.path:
    sys.path.insert(0, "/opt/trn_rl_repo")

import numpy as np

import concourse.bass as bass
import concourse.tile as tile
from concourse import bacc, mybir, library_config
from concourse.bass_utils import run_bass_kernel_spmd

# ---- problem constants (must match reference.py) ----
TAU = 0.07
THRESHOLD = 0.8
SAMPLE_NUM = 4096
CHUNK = 100
_EPS_NORM = 1e-12

N_CORES = 8
H = W = 512
HW = H * W
SHARD = HW // N_CORES  # 32768 pixels per core
C = 256
NA = SAMPLE_NUM  # anchors per class
ISL = NA // N_CORES  # 512 anchor i-slots per class per core
NFULL = NA // CHUNK  # 40 full chunks
NCHUNK = NFULL + 1  # 41 (incl. 96-negative remainder chunk)
CPAD = 3584  # padded per-class core-pixel count per core (28 * 128)
CBLK = 2 * CPAD // 128  # 56 gather-output blocks of 128 slots
GB = 8  # gather batches
BPB = CBLK // GB  # blocks per gather batch (7)

F32 = mybir.dt.float32
I16 = mybir.dt.int16
Alu = mybir.AluOpType
Act = mybir.ActivationFunctionType
Axis = mybir.AxisListType


# ---------------------------------------------------------------------------
# host-side plan: verbatim replica of reference._plan (numpy, seed 0)
# ---------------------------------------------------------------------------
def _plan(input_logits, input_seg, seed=0):
    logits = np.asarray(input_logits)
    seg = np.asarray(input_seg)
    gm = seg == 1
    bm = seg == 0
    gc = logits[:, 1] * gm
    bc = logits[:, 0] * bm
    mgc = float(gc.sum() / (gm.sum() + 1e-8))
    mbc = float(bc.sum() / (bm.sum() + 1e-8))
    rng = np.random.default_rng(seed)

    def samp(mask, num):
        coords = np.argwhere(mask)
        if len(coords) > num:
            coords = coords[rng.permutation(len(coords))[:num]]
        return coords

    easy_g = max(1, int(SAMPLE_NUM * (1 - mgc))); hard_g = SAMPLE_NUM - easy_g
    easy_b = max(1, int(SAMPLE_NUM * (1 - mbc))); hard_b = SAMPLE_NUM - easy_b
    ge = samp((gc >= mgc) & gm, easy_g)
    gh = samp((gc < mgc) & gm, hard_g)
    be = samp((bc >= mbc) & bm, easy_b)
    bh = samp((bc < mbc) & bm, hard_b)
    return {
        "g_anchor": np.concatenate([ge, gh]),
        "b_anchor": np.concatenate([be, bh]),
        "g_core": np.argwhere((gc >= THRESHOLD) & gm),
        "b_core": np.argwhere((bc >= THRESHOLD) & bm),
        "n_bg": len(be) + len(bh),
    }


# ---------------------------------------------------------------------------
# device kernel
# ---------------------------------------------------------------------------
def _build_kernel(do_loads=True, do_gather=True, do_coll=True, do_sim=True, nd=N_CORES):
    nc = bacc.Bacc("TRN2", target_bir_lowering=False, debug=False,
                   num_devices=nd)

    xp = nc.dram_tensor("xp", [SHARD, C], F32, kind="ExternalInput")
    cidx = nc.dram_tensor("cidx", [128, 2 * CPAD // 16], I16, kind="ExternalInput")
    cw = nc.dram_tensor("cw", [128, CBLK, 2], F32, kind="ExternalInput")
    amy = nc.dram_tensor("amy", [2, 128, 2 * ISL], F32, kind="ExternalInput")
    ball = nc.dram_tensor("ball", [2, 128, 2 * NA], F32, kind="ExternalInput")
    out = nc.dram_tensor("out", [128, 2], F32, kind="ExternalOutput")

    gsems = [nc.alloc_semaphore(f"gsem{t}") for t in range(GB)]

    with tile.TileContext(nc) as tc:
        with (
            tc.tile_pool(name="big", bufs=1) as big,
            tc.tile_pool(name="cg", bufs=2) as cgp,
            tc.tile_pool(name="esb", bufs=2) as esbp,
            tc.tile_pool(name="small", bufs=2) as small,
            tc.tile_pool(name="acc", bufs=1) as accp,
            tc.tile_pool(name="pe", bufs=3, space="PSUM") as pe_pool,
            tc.tile_pool(name="pm", bufs=1, space="PSUM") as pm_pool,
            tc.tile_pool(name="ps", bufs=2, space="PSUM") as ps_pool,
            tc.tile_pool(name="psq", bufs=1, space="PSUM") as psq_pool,
            tc.tile_pool(name="dram", bufs=1, space="DRAM") as dram,
        ):
            nc.gpsimd.load_library(library_config.attnmlp)

            partial = accp.tile([128, 2], F32, tag="partial")
            nc.vector.memset(partial[:], 0.0)

            # ---- resident inputs ----
            ball_sb = [big.tile([128, 2 * NA], F32, tag=f"ball{h}",
                                name=f"ball_sb{h}") for h in range(2)]
            amy_sb = [big.tile([128, 2 * ISL], F32, tag=f"amy{h}",
                               name=f"amy_sb{h}") for h in range(2)]
            cidx_sb = big.tile([128, 2 * CPAD // 16], I16, tag="cidx")
            cw_sb = big.tile([128, CBLK, 2], F32, tag="cw")
            if do_loads:
                for h in range(2):
                    nc.sync.dma_start(ball_sb[h][:], ball.ap()[h])
                    nc.sync.dma_start(amy_sb[h][:], amy.ap()[h])
                nc.sync.dma_start(cidx_sb[:], cidx.ap())
                nc.sync.dma_start(cw_sb[:], cw.ap())

            # ---- core-pixel gather + per-class mean partials ----
            mean_ps = [pm_pool.tile([128, 2], F32, tag=f"mean{h}",
                                    name=f"mean_ps{h}") for h in range(2)]
            mall = small.tile([128, 4], F32, tag="mall")  # h0g h0b h1g h1b
            if do_gather:
                nblk_total = 0
                for t in range(GB):
                    cg = cgp.tile([128, BPB, C], F32, tag="cg")
                    nidx = BPB * 128
                    nc.gpsimd.dma_gather(
                        out_ap=cg[:],
                        in_ap=xp.ap(),
                        idxs_ap=cidx_sb[:, t * (nidx // 16):(t + 1) * (nidx // 16)],
                        num_idxs=nidx,
                        num_idxs_reg=nidx,
                        elem_size=C,
                    ).then_inc(gsems[t], 16)
                    sq = cgp.tile([128, BPB, C], F32, tag="sq")
                    nc.scalar.activation(sq[:], cg[:], Act.Square)._wait_ge(
                        gsems[t], 16)
                    ssum = small.tile([128, BPB], F32, tag="ssum")
                    nc.vector.tensor_reduce(ssum[:], sq[:], Axis.X, Alu.add)
                    nrm = small.tile([128, BPB], F32, tag="nrm")
                    nc.scalar.activation(nrm[:], ssum[:], Act.Sqrt)
                    rnm = small.tile([128, BPB], F32, tag="rnm")
                    nc.vector.reciprocal(rnm[:], nrm[:])
                    w2 = small.tile([128, BPB, 2], F32, tag="w2")
                    for cls in range(2):
                        nc.vector.tensor_tensor(
                            w2[:, :, cls], cw_sb[:, t * BPB:(t + 1) * BPB, cls],
                            rnm[:], Alu.mult)
                    for b in range(BPB):
                        first = nblk_total == 0
                        last = nblk_total == CBLK - 1
                        for h in range(2):
                            nc.tensor.matmul(
                                mean_ps[h][:],
                                cg[:, b, h * 128:(h + 1) * 128],
                                w2[:, b, :],
                                start=first, stop=last,
                            )
                        nblk_total += 1

                # ---- AllReduce the mean partials ----
                msb = small.tile([128, 4], F32, tag="msb")
                for h in range(2):
                    nc.scalar.copy(msb[:, 2 * h:2 * h + 2], mean_ps[h][:])
                if do_coll:
                    mb_in = dram.tile([128, 4], F32, tag="mb_in")
                    mb_out = dram.tile([128, 4], F32, tag="mb_out")
                    nc.sync.dma_start(mb_in[:], msb[:])
                    nc.gpsimd.collective_compute(
                        "AllReduce", Alu.add,
                        replica_groups=[list(range(N_CORES))],
                        ins=[mb_in.opt()],
                        outs=[mb_out.opt()],
                    )
                    nc.sync.dma_start(mall[:], mb_out[:])
                else:
                    nc.vector.tensor_copy(mall[:], msb[:])
            else:
                nc.vector.memset(mall[:], 0.01)

            if do_sim:
                # ---- 1/||mean|| per class, broadcast columns ----
                sqn = psq_pool.tile([1, 2], F32, tag="sqn")
                for cls in range(2):
                    for h in range(2):
                        col = mall[:, 2 * h + cls:2 * h + cls + 1]
                        nc.tensor.matmul(sqn[:, cls:cls + 1], col, col,
                                         start=(h == 0), stop=(h == 1))
                rno = small.tile([1, 2], F32, tag="rno")
                nc.scalar.activation(rno[:], sqn[:], Act.Sqrt)
                rn = small.tile([1, 2], F32, tag="rn")
                nc.vector.reciprocal(rn[:], rno[:])
                rnb = small.tile([128, 2], F32, tag="rnb")
                nc.gpsimd.partition_broadcast(rnb[:], rn[:])
                c1 = small.tile([128, 2], F32, tag="c1")
                nc.scalar.mul(c1[:], rnb[:], -1.0 / TAU)

                # ---- contrastive part ----
                for cls in range(2):
                    joff = (1 - cls) * NA  # negatives = the other class
                    for ib in range(ISL // 128):
                        icol = cls * ISL + ib * 128
                        pos = ps_pool.tile([128, 1], F32, tag="pos")
                        for h in range(2):
                            nc.tensor.matmul(
                                pos[:],
                                amy_sb[h][:, icol:icol + 128],
                                mall[:, 2 * h + cls:2 * h + cls + 1],
                                start=(h == 0), stop=(h == 1),
                            )
                        esb = esbp.tile([128, NA], F32, tag="esb")
                        for js in range(NA // 512):
                            eps = pe_pool.tile([128, 512], F32, tag="eps")
                            for h in range(2):
                                nc.tensor.matmul(
                                    eps[:],
                                    amy_sb[h][:, icol:icol + 128],
                                    ball_sb[h][:, joff + js * 512:
                                               joff + (js + 1) * 512],
                                    start=(h == 0), stop=(h == 1),
                                )
                            nc.scalar.activation(
                                esb[:, js * 512:(js + 1) * 512], eps[:],
                                Act.Exp, scale=1.0 / TAU)
                        r_all = small.tile([128, NCHUNK], F32, tag="r_all")
                        nc.vector.tensor_reduce(
                            r_all[:, 0:NFULL],
                            esb[:, 0:NFULL * CHUNK].rearrange(
                                "p (a b) -> p a b", b=CHUNK),
                            Axis.X, Alu.add)
                        nc.vector.tensor_reduce(
                            r_all[:, NFULL:NCHUNK],
                            esb[:, NFULL * CHUNK:NA], Axis.X, Alu.add)
                        eposn = small.tile([128, 1], F32, tag="eposn")
                        nc.scalar.activation(eposn[:], pos[:], Act.Exp,
                                             scale=c1[:, cls:cls + 1])
                        sprime = small.tile([128, NCHUNK], F32, tag="sprime")
                        nc.vector.tensor_scalar_mul(sprime[:], r_all[:],
                                                    eposn[:, 0:1])
                        lchunk = small.tile([128, NCHUNK], F32, tag="lchunk")
                        nc.scalar.activation(lchunk[:], sprime[:], Act.Ln,
                                             bias=1.0)
                        lcol = small.tile([128, 1], F32, tag="lcol")
                        nc.vector.tensor_reduce(lcol[:], lchunk[:], Axis.X,
                                                Alu.add)
                        nc.vector.tensor_tensor(
                            partial[:, cls:cls + 1], partial[:, cls:cls + 1],
                            lcol[:], Alu.add)

            nc.sync.dma_start(out.ap(), partial[:])

    nc.compile()
    return nc


_NC_CACHE = None


def _get_nc():
    global _NC_CACHE
    if _NC_CACHE is None:
        _NC_CACHE = _build_kernel()
    return _NC_CACHE


# ---------------------------------------------------------------------------
# host orchestration
# ---------------------------------------------------------------------------
def _wrap_idx(idx_flat):
    """int16 flat index list -> dma_gather layout [128, n/16]."""
    n = len(idx_flat)
    arr = np.asarray(idx_flat, np.int16).reshape(n // 16, 16).T  # [16, n/16]
    return np.tile(arr, (8, 1))  # replicate to 128 partitions


def _prep_inputs(input, input_logits, input_seg):
    x = np.asarray(input)
    plan = _plan(input_logits, input_seg)
    assert len(plan["g_anchor"]) == NA and len(plan["b_anchor"]) == NA
    assert plan["n_bg"] == NA

    x2d = np.ascontiguousarray(x.reshape(C, HW))

    # ---- anchors: host gather + normalize (fp32), channel-major global order
    def anchors_chmaj(coords):
        p = coords[:, 1] * W + coords[:, 2]
        a = x2d[:, p].T.astype(np.float32)  # [NA, C]
        n = np.sqrt((a * a).sum(axis=1, dtype=np.float32))
        a /= np.maximum(n, _EPS_NORM)[:, None]
        return a.T  # [C, NA]

    ag = anchors_chmaj(plan["g_anchor"])
    ab = anchors_chmaj(plan["b_anchor"])
    ball_np = np.empty((2, 128, 2 * NA), np.float32)
    for h in range(2):
        ball_np[h, :, :NA] = ag[h * 128:(h + 1) * 128]
        ball_np[h, :, NA:] = ab[h * 128:(h + 1) * 128]

    # ---- per-core tensors
    in_maps = []
    pg = plan["g_core"][:, 1] * W + plan["g_core"][:, 2]
    pb = plan["b_core"][:, 1] * W + plan["b_core"][:, 2]
    ngc, nbc = len(pg), len(pb)
    for k in range(N_CORES):
        lo = k * SHARD
        xp_k = np.ascontiguousarray(x2d[:, lo:lo + SHARD].T)  # [SHARD, C]

        idx = np.zeros(2 * CPAD, np.int16)
        w = np.zeros((2, 2 * CPAD), np.float32)
        for cls, (p_all, ntot) in enumerate(((pg, ngc), (pb, nbc))):
            pl = p_all[(p_all >= lo) & (p_all < lo + SHARD)] - lo
            assert len(pl) <= CPAD, f"core {k} class {cls}: {len(pl)} > {CPAD}"
            idx[cls * CPAD:cls * CPAD + len(pl)] = pl.astype(np.int16)
            w[cls, cls * CPAD:cls * CPAD + len(pl)] = 1.0 / ntot
        cidx_np = _wrap_idx(idx)
        # cw layout matches gather output: slot s -> [s%128, s//128, cls]
        cw_np = np.ascontiguousarray(
            w.reshape(2, CBLK, 128).transpose(2, 1, 0)).astype(np.float32)

        amy_np = np.empty((2, 128, 2 * ISL), np.float32)
        for h in range(2):
            amy_np[h, :, :ISL] = ball_np[h, :, k * ISL:(k + 1) * ISL]
            amy_np[h, :, ISL:] = ball_np[h, :, NA + k * ISL:NA + (k + 1) * ISL]

        in_maps.append({
            "xp": xp_k,
            "cidx": cidx_np,
            "cw": cw_np,
            "amy": amy_np,
            "ball": ball_np,
        })
    return in_maps


def kernel(input, input_logits, input_seg):
    nc = _get_nc()
    in_maps = _prep_inputs(input, input_logits, input_seg)
    res = run_bass_kernel_spmd(nc, in_maps, list(range(N_CORES)))
    tot = np.zeros(2, np.float64)
    for k in range(N_CORES):
        tot += res.results[k]["out"].astype(np.float64).sum(axis=0)
    loss = (tot[0] + tot[1]) / (NCHUNK * NA)
    return np.float32(loss)

